# revision 1
# baseline (speedup 1.0000x reference)
"""Trainium2 Bass kernel for nn_BaseAtt (attention pooling).

reference:
    target = target_feats @ W.T                      # [B, 128]
    alpha  = softmax(mask(nf @ target), axis=k)      # [B, 200]
    onf    = sum_k alpha * nf                        # [B, 128]
    onl    = sum_k alpha * nl                        # [B, 128]

Sharding: data-parallel over B across 8 cores (512 batches/core).

Per-core pipeline (block = 32 batches, DMA group = 16 batches):
  - target.T [128d, 512b] via 8 accumulated fp32r matmuls (host-transposed
    W.T and target_feats.T inputs).
  - per batch: PE-transpose nf[b] (k-part layout, exact fp32) -> nfT
    [128d, 200k] in SBUF.
  - alpha rows: per-batch matmul with zero-masked stationary columns
    (z[:, i, :] = targetT col i on diag col i, else 0) accumulating into one
    PSUM tile [32, 256] -> all 32 alpha rows land on partitions 0..31.
  - standard softmax in b-partition layout.
  - weighted sums: same masked-stationary trick with alpha.T columns against
    the k-partition nf|nl tiles, accumulating [32, 256] output rows.
Blocks are software-pipelined (alpha of block bb issues before weighted of
bb-1) so the PE never idles across the softmax dependency, keeping the HAM
clock-gate warm.  Big DMA loads alternate between the sync and scalar HWDGE
rings; small transfers ride SWDGE (gpsimd).
"""

import numpy as np

B, K, D, FD = 4096, 200, 128, 1024
NCORES = 8
BC = B // NCORES          # 512 batches per core
BLK = 16                  # softmax / MM-accumulation / DMA block
K0, K1 = 128, K - 128     # k-chunk sizes (128 + 72)
NEG = -1.0e9              # mask fill for logits (exp -> 0)


def gen_kernel():
    import concourse.bacc as bacc
    import concourse.tile as tile
    from concourse import mybir

    f32 = mybir.dt.float32
    f32r = mybir.dt.float32r
    f16 = mybir.dt.float16
    AX = mybir.AxisListType
    AF = mybir.ActivationFunctionType

    nc = bacc.Bacc()

    tft = nc.declare_dram_parameter("tft", [FD, BC], f32r, isOutput=False)
    wt = nc.declare_dram_parameter("wt", [FD, D], f32r, isOutput=False)
    xh = nc.declare_dram_parameter("xh", [K, BC, 2 * D], f16, isOutput=False)
    nfth = nc.declare_dram_parameter("nfth", [D, BC, K], f32r, isOutput=False)
    lmask = nc.declare_dram_parameter("lmask", [BC, K], f32, isOutput=False)
    ident = nc.declare_dram_parameter("ident", [128, 128], f32r, isOutput=False)
    m32 = nc.declare_dram_parameter("m32", [128, BLK, BLK], f32r, isOutput=False)
    m32h = nc.declare_dram_parameter("m32h", [128, BLK, BLK], f16, isOutput=False)

    onf = nc.declare_dram_parameter("onf", [BC, D], f32, isOutput=True)
    onl = nc.declare_dram_parameter("onl", [BC, D], f32, isOutput=True)

    with tile.TileContext(nc) as tc:
        with (
            tc.tile_pool(name="const", bufs=1) as const,
            tc.tile_pool(name="xin", bufs=3) as xin,
            tc.tile_pool(name="nftile", bufs=3) as nftp,
            tc.tile_pool(name="sm", bufs=3) as sm,
            tc.tile_pool(name="lmp", bufs=4) as lmp,
            tc.tile_pool(name="zp", bufs=2) as zp,
            tc.tile_pool(name="outp", bufs=4) as outp,
            tc.tile_pool(name="pst", bufs=1, space="PSUM") as pst,
            tc.tile_pool(name="psa", bufs=2, space="PSUM") as psa,
            tc.tile_pool(name="pso", bufs=2, space="PSUM") as pso,
            tc.tile_pool(name="psx", bufs=1, space="PSUM") as psx,
        ):
            # ---- setup: constants ----
            id_t = const.tile([128, 128], f32r)
            nc.gpsimd.dma_start(out=id_t, in_=ident[:, :])
            m32_t = const.tile([128, BLK, BLK], f32r)
            nc.gpsimd.dma_start(out=m32_t, in_=m32[:, :, :])
            m32h_t = const.tile([128, BLK, BLK], f16)
            nc.gpsimd.dma_start(out=m32h_t, in_=m32h[:, :, :])
            with tc.tile_pool(name="setup", bufs=1) as setup:
                wt_t = setup.tile([128, 8, D], f32r)
                nc.scalar.dma_start(
                    out=wt_t, in_=wt.rearrange("(fb fp) d -> fp fb d", fp=128)
                )
                tft_t = setup.tile([128, 8, BC], f32r)
                nc.scalar.dma_start(
                    out=tft_t, in_=tft.rearrange("(fb fp) b -> fp fb b", fp=128)
                )

                # ---- target.T = W @ tf.T : [128 d, BC b] ----
                ps_t = pst.tile([128, BC], f32)
                for fb in range(8):
                    nc.tensor.matmul(
                        ps_t, wt_t[:, fb, :], tft_t[:, fb, :],
                        start=(fb == 0), stop=(fb == 7),
                    )
                targetT = const.tile([128, BC], f32r)
                nc.vector.tensor_copy(out=targetT, in_=ps_t.bitcast(f32r))

            def load_x(bb):
                """Issue the big x-tile loads for block bb.

                x0 rides the sync HWDGE ring, x1 rides SWDGE (gpsimd); the
                host packs xh k-major so each partition reads one contiguous
                16 KB run.
                """
                b0 = bb * BLK
                x0 = xin.tile([128, BLK, 256], f16, tag="x0")
                x1 = xin.tile([K1, BLK, 256], f16, tag="x1")
                # flat nfT tile: per-batch windows [i*K, i*K+256) overlap into
                # the next batch (junk cols land in out[:, 200:256], ignored)
                nft_t = nftp.tile([128, BLK * K + 64], f32r, tag="nft")
                nc.sync.dma_start(out=x0, in_=xh[0:K0, b0 : b0 + BLK, :])
                nc.gpsimd.dma_start(out=x1, in_=xh[K0:K, b0 : b0 + BLK, :])
                nc.scalar.dma_start(
                    out=nft_t[:, 0 : BLK * K],
                    in_=nfth[:, b0 : b0 + BLK, :].rearrange("d b k -> d (b k)"),
                )
                return x0, x1, nft_t

            def build_z(bb):
                b0 = bb * BLK
                z_t = zp.tile([128, BLK, BLK], f32r, tag="z")
                nc.vector.tensor_mul(
                    out=z_t,
                    in0=targetT[:, b0 : b0 + BLK].unsqueeze(2).broadcast_to(
                        [128, BLK, BLK]
                    ),
                    in1=m32_t,
                )
                return z_t

            def alpha_phase(bb, z_t, sm_prev, xload=None):
                """DMA loads, per-batch nf transposes, alpha-row matmuls.

                Injects the previous block's alpha.T/za prep into the middle
                of this block's PE stream so za is ready (built on DVE in the
                shadow of these matmuls) by the time weighted_phase issues.
                """
                b0 = bb * BLK
                lm_t = lmp.tile([BLK, K], f32, tag="lm")
                nc.scalar.dma_start(out=lm_t, in_=lmask[b0 : b0 + BLK, :])
                ps_a = psa.tile([BLK, 256], f32, tag="psa")
                x0, x1, nft_t = xload if xload is not None else load_x(bb)
                w_ready = None
                for i in range(BLK):
                    if i == 2 and sm_prev is not None:
                        w_ready = prep_weighted(sm_prev)
                    # alpha row i accumulates into ps_a (host-transposed nfT)
                    nc.tensor.matmul(
                        ps_a, z_t[:, i, :], nft_t[:, i * K : i * K + 256],
                        start=(i == 0), stop=(i == BLK - 1),
                    )
                if sm_prev is not None and w_ready is None:
                    w_ready = prep_weighted(sm_prev)
                return b0, ps_a, x0, x1, lm_t, w_ready

            def softmax_phase(state):
                """Softmax arithmetic on DVE/ACT only (no PE instructions)."""
                b0, ps_a, x0, x1, lm_t = state
                aM = sm.tile([BLK, K], f32, tag="am")
                nc.vector.tensor_add(out=aM, in0=ps_a[:, 0:K], in1=lm_t)
                mx = sm.tile([BLK, 1], f32, tag="mx")
                nc.vector.reduce_max(out=mx, in_=aM, axis=AX.X)
                negmx = sm.tile([BLK, 1], f32, tag="negmx")
                nc.vector.tensor_scalar_mul(out=negmx, in0=mx, scalar1=-1.0)
                aE = sm.tile([BLK, K], f32, tag="ae")
                s_t = sm.tile([BLK, 1], f32, tag="s")
                nc.scalar.activation(
                    out=aE, in_=aM, func=AF.Exp, bias=negmx, scale=1.0,
                    accum_out=s_t,
                )
                rs = sm.tile([BLK, 1], f32, tag="rs")
                nc.vector.reciprocal(out=rs, in_=s_t)
                aN = sm.tile([BLK, K], f32r, tag="an")
                nc.vector.tensor_scalar_mul(out=aN, in0=aE, scalar1=rs)
                return b0, aN, x0, x1

            def prep_weighted(smstate):
                """alpha.T PE transposes + za stationary builds."""
                b0, aN, x0, x1 = smstate
                # alpha.T via PE transpose: [200 k, 16 b]
                ps_aT = psx.tile([128, 2 * BLK], f32r, tag="pat")
                nc.tensor.transpose(ps_aT[:, 0:BLK], aN[:, 0:K0], id_t[:BLK, :BLK])
                nc.tensor.transpose(
                    ps_aT[:K1, BLK : 2 * BLK], aN[:, K0:K], id_t[:BLK, :BLK]
                )
                aT0 = zp.tile([128, BLK], f16, tag="at0")
                nc.scalar.copy(out=aT0, in_=ps_aT[:, 0:BLK])
                aT1 = zp.tile([K1, BLK], f16, tag="at1")
                nc.vector.tensor_copy(out=aT1, in_=ps_aT[:K1, BLK : 2 * BLK])

                za0 = zp.tile([128, BLK, BLK], f16, tag="za0")
                nc.vector.tensor_mul(
                    out=za0,
                    in0=aT0.unsqueeze(2).broadcast_to([128, BLK, BLK]),
                    in1=m32h_t,
                )
                za1 = zp.tile([K1, BLK, BLK], f16, tag="za1")
                nc.vector.tensor_mul(
                    out=za1,
                    in0=aT1.unsqueeze(2).broadcast_to([K1, BLK, BLK]),
                    in1=m32h_t[:K1],
                )
                return b0, x0, x1, za0, za1

            def weighted_phase(wstate):
                """Weighted-sum matmuls + output copy/DMA."""
                b0, x0, x1, za0, za1 = wstate
                ps_o = pso.tile([BLK, 256], f32, tag="pso")
                for i in range(BLK):
                    nc.tensor.matmul(
                        ps_o, za0[:, i, :], x0[:, i, :],
                        start=(i == 0), stop=False,
                    )
                    nc.tensor.matmul(
                        ps_o, za1[:, i, :], x1[:, i, :],
                        start=False, stop=(i == BLK - 1),
                    )
                out_s = outp.tile([BLK, 256], f32, tag="outs")
                nc.vector.tensor_copy(out=out_s[:, 0:128], in_=ps_o[:, 0:128])
                nc.scalar.copy(out=out_s[:, 128:256], in_=ps_o[:, 128:256])
                nc.scalar.dma_start(
                    out=onf[b0 : b0 + BLK, :], in_=out_s[:, 0:128]
                )
                nc.scalar.dma_start(
                    out=onl[b0 : b0 + BLK, :], in_=out_s[:, 128:256]
                )

            # ---- software-pipelined main loop ----
            sm_prev = None
            z_next = build_z(0)
            xpre = {0: load_x(0), 1: load_x(1)}
            for bb in range(BC // BLK):
                z_cur = z_next
                b0, ps_a, x0, x1, lm_t, w_ready = alpha_phase(
                    bb, z_cur, sm_prev, xload=xpre.pop(bb, None)
                )
                if bb + 1 < BC // BLK:
                    z_next = build_z(bb + 1)
                if w_ready is not None:
                    weighted_phase(w_ready)
                sm_prev = softmax_phase((b0, ps_a, x0, x1, lm_t))
            weighted_phase(prep_weighted(sm_prev))

    nc.finalize()
    return nc


_NC_CACHE = None


def _get_nc():
    global _NC_CACHE
    if _NC_CACHE is None:
        _NC_CACHE = gen_kernel()
    return _NC_CACHE


def build_in_maps(target_feats, neighbor_feats, neighbor_label, hist_mask, W):
    target_feats = np.ascontiguousarray(target_feats, dtype=np.float32)
    neighbor_feats = np.ascontiguousarray(neighbor_feats, dtype=np.float32)
    neighbor_label = np.ascontiguousarray(neighbor_label, dtype=np.float32)
    W = np.ascontiguousarray(W, dtype=np.float32)

    wt_full = np.ascontiguousarray(W.T)                      # [FD, D]
    lmask_full = np.where(np.asarray(hist_mask) > 0, 0.0, NEG).astype(np.float32)
    ident = np.eye(128, dtype=np.float32)
    m32 = np.zeros((128, BLK, BLK), dtype=np.float32)
    for i in range(BLK):
        m32[:, i, i] = 1.0

    in_maps = []
    for c in range(NCORES):
        s = slice(c * BC, (c + 1) * BC)
        # k-major interleaved nf|nl: xh[k, b, 0:128]=nf, xh[k, b, 128:256]=nl
        xh = np.empty((K, BC, 2 * D), dtype=np.float16)
        xh[:, :, 0:D] = neighbor_feats[s].transpose(1, 0, 2)
        xh[:, :, D : 2 * D] = neighbor_label[s].transpose(1, 0, 2)
        nfth = np.ascontiguousarray(neighbor_feats[s].transpose(2, 0, 1))
        in_maps.append({
            "tft": np.ascontiguousarray(target_feats[s].T),  # [FD, BC]
            "wt": wt_full,
            "xh": xh,
            "nfth": nfth,
            "lmask": lmask_full[s],
            "ident": ident,
            "m32": m32,
            "m32h": m32.astype(np.float16),
        })
    return in_maps


def kernel(target_feats, neighbor_feats, neighbor_label, hist_mask, W):
    from concourse.bass_utils import run_bass_kernel_spmd

    in_maps = build_in_maps(
        target_feats, neighbor_feats, neighbor_label, hist_mask, W
    )
    nc = _get_nc()
    res = run_bass_kernel_spmd(nc, in_maps, list(range(NCORES))).results

    onf = np.concatenate([res[c]["onf"] for c in range(NCORES)], axis=0)
    onl = np.concatenate([res[c]["onl"] for c in range(NCORES)], axis=0)
    return onf, onl

